# revision 10
# baseline (speedup 1.0000x reference)
"""Trainium2 Bass kernel for DOAModel correlation + dual-softmax + top-k.

Per batch (reference semantics, no softmax max-subtraction needed since
|logits| <= alpha * max|corr| which is small):
  a    = alpha * l2norm_c(xp)^T l2norm_c(xq)            [N, N]
  x_c  = exp(2a) / (rowsum(exp a)[p] * colsum(exp a)[q])
       = (h1[p] * h2[q] * exp(a))^2,  h = rsqrt(sum exp a)
  valp/valq = top-20 of rows / cols of x_c (sorted), plus x_c and x_c^T.

Implementation notes:
  - matmul in float32r (tf32-like, 1 cyc/row for N>=256): rel err ~1e-4
  - exp on ScalarE with per-partition scale (folds rp) and accum_out
    giving row sums for free; E stored bf16
  - E' = E * h1 on GPSIMD; col sums via one-shot PE matmuls + DVE adds
  - x_c = Square(E' (x) bcast(h2)); x_c^T tiles via PE transpose of E'
    blocks then Square(scale=h2-chunk)
  - top-20 = 3x nc.vector.max + 2x nc.vector.match_replace per tile
Sharding: data-parallel over batch; 16 batches -> 8 cores x 2.
"""

import sys

sys.path.insert(0, "/opt/trn_rl_repo")

import numpy as np

import concourse.bacc as bacc
import concourse.mybir as mybir
import concourse.tile as tile
from concourse.masks import make_identity
from concourse.bass_utils import run_bass_kernel_spmd

dt = mybir.dt
AF = mybir.ActivationFunctionType
ALU = mybir.AluOpType
AX = mybir.AxisListType

C = 896           # channels (contraction dim)
KC = C // 128     # 7 k-chunks
N = 1600          # h*w
NT = 13           # row tiles: 12 x 128 + 1 x 64
NCH = 400         # matmul free-dim chunk (<=512 per matmul, >=256 keeps f32r fast)
ECH = 800         # exp chunk (2 PSUM banks; 2 matmul sub-chunks)
ECHN = N // ECH   # 2
NCHN = N // NCH   # 4
TOPK = 20
NEG = -1.0        # match_replace fill; x_c >= 0

M_OFF = [i * 128 for i in range(NT)]
M_SZ = [128] * 12 + [64]


def _ms(i):
    return slice(M_OFF[i], M_OFF[i] + M_SZ[i])


def build(alpha: float, n_batches: int):
    nc = bacc.Bacc("TRN2", target_bir_lowering=False, debug=False)
    B = n_batches
    xp_d = nc.declare_dram_parameter("xp", [B, C, N], dt.float32, isOutput=False)
    xq_d = nc.declare_dram_parameter("xq", [B, C, N], dt.float32, isOutput=False)
    valp_d = nc.declare_dram_parameter("valp", [B, TOPK, N], dt.float32, isOutput=True)
    valq_d = nc.declare_dram_parameter("valq", [B, TOPK, N], dt.float32, isOutput=True)
    xcoq_d = nc.declare_dram_parameter("xc_o_q", [B, N, N], dt.float32, isOutput=True)
    xcop_d = nc.declare_dram_parameter("xc_o_p", [B, N, N], dt.float32, isOutput=True)

    from contextlib import ExitStack

    with tile.TileContext(nc) as tc, ExitStack() as ctx:
        p_const = ctx.enter_context(tc.tile_pool(name="const", bufs=1))
        p_f32 = ctx.enter_context(tc.tile_pool(name="f32s", bufs=18))
        p_b16 = ctx.enter_context(tc.tile_pool(name="b16s", bufs=17))
        p_sqq = ctx.enter_context(tc.tile_pool(name="sqq", bufs=3))
        p_row = ctx.enter_context(tc.tile_pool(name="rows", bufs=1))
        p_sml = ctx.enter_context(tc.tile_pool(name="sml", bufs=4))
        ps_a = ctx.enter_context(tc.tile_pool(name="psA", bufs=4, space="PSUM"))
        ps_t = ctx.enter_context(tc.tile_pool(name="psT", bufs=2, space="PSUM"))
        ps_r = ctx.enter_context(tc.tile_pool(name="psR", bufs=1, space="PSUM"))
        ps_s = ctx.enter_context(tc.tile_pool(name="psS", bufs=1, space="PSUM"))

        identb = p_const.tile([128, 128], dt.bfloat16)
        make_identity(nc, identb[:])
        identf = p_const.tile([128, 128], dt.float32)
        make_identity(nc, identf[:])
        ones_b = p_const.tile([128, 1], dt.bfloat16)
        nc.vector.memset(ones_b[:], 1.0)

        for b in range(B):
            # =========================================================
            # Phase 0: load, norms, Bq, scale xq in place
            # =========================================================
            xp_t = []
            for k in range(KC):
                t = p_f32.tile([128, N], dt.float32r, tag="big")
                nc.sync.dma_start(
                    t[:], xp_d[b, k * 128:(k + 1) * 128, :].bitcast(dt.float32r))
                xp_t.append(t)
            xq_t = []
            for k in range(KC):
                t = p_f32.tile([128, N], dt.float32, tag="big")
                nc.sync.dma_start(t[:], xq_d[b, k * 128:(k + 1) * 128, :])
                xq_t.append(t)

            # --- sumsq_p chunk-form [128, NT]: one-shot matmuls + SBUF acc
            ssp = p_sml.tile([128, 16], dt.float32, tag="sm_a")
            nc.vector.memset(ssp[:], 0.0)
            for k in range(KC):
                sq = p_b16.tile([128, N], dt.bfloat16, tag="bb")
                nc.scalar.activation(sq[:], xp_t[k][:].bitcast(dt.float32), AF.Square)
                ps = ps_s.tile([128, 16], dt.float32, tag="small")
                for m in range(NT):
                    nc.tensor.matmul(ps[:M_SZ[m], m:m + 1], sq[:, _ms(m)], ones_b[:],
                                     start=(m == 0), stop=(m == NT - 1))
                nc.vector.tensor_tensor(ssp[:, :NT], ssp[:, :NT], ps[:, :NT], ALU.add)
            rp_r = p_sml.tile([128, 16], dt.float32, tag="sm_b")
            nc.vector.reciprocal(rp_r[:, :NT], ssp[:, :NT])
            rpc = p_sml.tile([128, 16], dt.float32, tag="sm_c")
            nc.scalar.activation(rpc[:, :NT], rp_r[:, :NT], AF.Sqrt)

            # --- sumsq_q row-form [1, N] (for Bq): ones^T @ xq^2
            rq_row = p_row.tile([1, N], dt.float32, tag="row_f")
            for ch in range(NCHN):
                ssq = ps_r.tile([1, NCH], dt.float32, tag="rowp")
                for k in range(KC):
                    sq = p_sqq.tile([128, NCH], dt.bfloat16, tag="sqq")
                    nc.scalar.activation(
                        sq[:], xq_t[k][:, ch * NCH:(ch + 1) * NCH], AF.Square)
                    nc.tensor.matmul(ssq[:], ones_b[:], sq[:],
                                     start=(k == 0), stop=(k == KC - 1))
                rec = p_sml.tile([1, NCH], dt.float32, tag="sm_d")
                nc.vector.reciprocal(rec[:], ssq[:])
                # alpha * rsqrt(s) = Sqrt(alpha^2 / s)
                nc.scalar.activation(rq_row[:, ch * NCH:(ch + 1) * NCH], rec[:],
                                     AF.Sqrt, scale=float(alpha) * float(alpha))

            bq = p_f32.tile([128, N], dt.float32, tag="big")
            nc.gpsimd.partition_broadcast(bq[:], rq_row[:])
            # xqs = xq * Bq, rounded to f32r (gpsimd); xq slot recycles
            xqs_t = []
            for k in range(KC):
                t = p_f32.tile([128, N], dt.float32r, tag="big")
                nc.vector.tensor_tensor(t[:], xq_t[k][:], bq[:], ALU.mult)
                xqs_t.append(t)

            # =========================================================
            # Phase 1: per row-tile matmul + exp + E' + col sums
            # =========================================================
            ep_t = []
            s2sb = p_sml.tile([128, 16], dt.float32, tag="sm_e")
            nc.vector.memset(s2sb[:], 0.0)
            s1acc = p_sml.tile([128, 64], dt.float32, tag="sm_f")
            for m in range(NT):
                msz = M_SZ[m]
                e_t = p_b16.tile([128, N], dt.bfloat16, tag="bb")
                if msz < 128:
                    nc.vector.memset(e_t[:], 0.0)
                for ch in range(NCHN):
                    pa = ps_a.tile([128, NCH], dt.float32, tag="A")
                    for k in range(KC):
                        nc.tensor.matmul(
                            pa[:msz, :],
                            xp_t[k][:, _ms(m)],
                            xqs_t[k][:, ch * NCH:(ch + 1) * NCH],
                            start=(k == 0), stop=(k == KC - 1))
                    nc.scalar.activation(
                        e_t[:msz, ch * NCH:(ch + 1) * NCH], pa[:msz, :], AF.Exp,
                        scale=rpc[:msz, m:m + 1],
                        accum_out=s1acc[:msz, 4 * m + ch:4 * m + ch + 1])
                ep_t.append(e_t)

                # col-sum contribution of this row tile (one-shot + SBUF acc)
                ps2 = ps_s.tile([128, 16], dt.float32, tag="small")
                for j in range(NT):
                    nc.tensor.matmul(ps2[:M_SZ[j], j:j + 1], e_t[:, _ms(j)], ones_b[:],
                                     start=(j == 0), stop=(j == NT - 1))
                nc.vector.tensor_tensor(s2sb[:, :NT], s2sb[:, :NT], ps2[:, :NT], ALU.add)

            # =========================================================
            # Phase 1b: batched h1 (rows) + h2 (cols), one table load each
            # =========================================================
            s1 = p_sml.tile([128, 16], dt.float32, tag="sm_g")
            nc.vector.reduce_sum(s1[:, :NT],
                                 s1acc[:, :4 * NT].rearrange("p (m c) -> p m c", c=4),
                                 axis=AX.X)
            h1r = p_sml.tile([128, 16], dt.float32, tag="sm_h")
            nc.vector.reciprocal(h1r[:, :NT], s1[:, :NT])
            h1c = p_sml.tile([128, 16], dt.float32, tag="sm_i")
            nc.scalar.activation(h1c[:, :NT], h1r[:, :NT], AF.Sqrt)
            h2r_ = p_sml.tile([128, 16], dt.float32, tag="sm_j")
            nc.vector.reciprocal(h2r_[:, :NT], s2sb[:, :NT])
            h2c = p_sml.tile([128, 16], dt.float32, tag="sm_k")
            nc.scalar.activation(h2c[:, :NT], h2r_[:, :NT], AF.Sqrt)
            h2row = p_row.tile([1, N], dt.bfloat16, tag="row_b")
            for j in range(NT):
                pt = ps_t.tile([1, 128], dt.float32, tag="T")
                nc.tensor.transpose(pt[:, :M_SZ[j]], h2c[:M_SZ[j], j:j + 1],
                                    identf[:M_SZ[j], :M_SZ[j]])
                nc.vector.tensor_copy(h2row[:, _ms(j)], pt[:, :M_SZ[j]])
            bs2h = p_b16.tile([128, N], dt.bfloat16, tag="bb")
            nc.gpsimd.partition_broadcast(bs2h[:], h2row[:])

            # Phase 1c: E' = E * h1 in place (DVE tensor_scalar, bf16 4x)
            for m in range(NT):
                nc.vector.tensor_scalar_mul(ep_t[m][:M_SZ[m], :], ep_t[m][:M_SZ[m], :],
                                            h1c[:M_SZ[m], m:m + 1])

            # =========================================================
            # Phase 2b: x_c tiles (rows) and x_c^T tiles (cols) + top-k
            # =========================================================
            valp_row = p_f32.tile([32, N], dt.float32, tag="big")
            valq_row = p_f32.tile([32, N], dt.float32, tag="big")

            def topk_into(xc, msz, vrow, m):
                vals = p_sml.tile([128, 24], dt.float32, tag="sm_l")
                scr = p_f32.tile([128, N], dt.float32, tag="big")
                scr2 = p_f32.tile([128, N], dt.float32, tag="big")
                nc.vector.max(out=vals[:msz, 0:8], in_=xc[:msz, :])
                nc.vector.match_replace(out=scr[:msz, :], in_to_replace=vals[:msz, 0:8],
                                        in_values=xc[:msz, :], imm_value=NEG)
                nc.vector.max(out=vals[:msz, 8:16], in_=scr[:msz, :])
                nc.vector.match_replace(out=scr2[:msz, :], in_to_replace=vals[:msz, 8:16],
                                        in_values=scr[:msz, :], imm_value=NEG)
                nc.vector.max(out=vals[:msz, 16:24], in_=scr2[:msz, :])
                pt = ps_t.tile([32, 128], dt.float32, tag="T")
                nc.tensor.transpose(pt[:TOPK, :msz], vals[:msz, :TOPK],
                                    identf[:msz, :msz])
                nc.vector.tensor_copy(vrow[:TOPK, _ms(m)], pt[:TOPK, :msz])

            for t in range(NT):
                msz = M_SZ[t]
                # rows side: x_c tile t
                g = p_b16.tile([128, N], dt.bfloat16, tag="bb")
                nc.vector.tensor_tensor(g[:msz, :], ep_t[t][:msz, :], bs2h[:msz, :],
                                        ALU.mult)
                xc = p_f32.tile([128, N], dt.float32, tag="big")
                nc.scalar.activation(xc[:msz, :], g[:msz, :], AF.Square)
                nc.sync.dma_start(xcoq_d[b, _ms(t), :], xc[:msz, :])
                topk_into(xc, msz, valp_row, t)

                # cols side: x_c^T tile t from 13 transposed blocks
                xct = p_f32.tile([128, N], dt.float32, tag="big")
                for i in range(NT):
                    pt = ps_t.tile([128, 128], dt.bfloat16, tag="T")
                    nc.tensor.transpose(pt[:msz, :], ep_t[i][:, _ms(t)], identb[:])
                    nc.scalar.activation(xct[:msz, _ms(i)], pt[:msz, :M_SZ[i]],
                                         AF.Square, scale=h2c[:msz, t:t + 1])
                nc.sync.dma_start(xcop_d[b, _ms(t), :], xct[:msz, :])
                topk_into(xct, msz, valq_row, t)

            nc.sync.dma_start(valp_d[b], valp_row[:TOPK, :])
            nc.sync.dma_start(valq_d[b], valq_row[:TOPK, :])

    nc.compile()
    return nc


_CACHE = {}


def _get_kernel(alpha: float, n_batches: int):
    key = (round(float(alpha), 6), n_batches)
    if key not in _CACHE:
        _CACHE[key] = build(float(alpha), n_batches)
    return _CACHE[key]


def kernel(xp, xq, alpha):
    xp = np.asarray(xp, dtype=np.float32)
    xq = np.asarray(xq, dtype=np.float32)
    b, c, h1, w1 = xp.shape
    _, _, h2, w2 = xq.shape
    n_cores = 8
    assert b % n_cores == 0
    bpc = b // n_cores
    nc = _get_kernel(float(alpha), bpc)

    xp_f = xp.reshape(b, c, h1 * w1)
    xq_f = xq.reshape(b, c, h2 * w2)
    in_maps = [
        {"xp": xp_f[i * bpc:(i + 1) * bpc], "xq": xq_f[i * bpc:(i + 1) * bpc],
         }
        for i in range(n_cores)
    ]
    res = run_bass_kernel_spmd(nc, in_maps, list(range(n_cores)))
    valp = np.concatenate([r["valp"] for r in res.results], axis=0)
    valq = np.concatenate([r["valq"] for r in res.results], axis=0)
    xcop = np.concatenate([r["xc_o_p"] for r in res.results], axis=0)
    xcoq = np.concatenate([r["xc_o_q"] for r in res.results], axis=0)
    return (
        valp.reshape(b, TOPK, h1, w1),
        valq.reshape(b, TOPK, h2, w2),
        xcop.reshape(b, h2 * w2, h1, w1),
        xcoq.reshape(b, h1 * w1, h2, w2),
    )


# revision 13
# speedup vs baseline: 1.0270x; 1.0270x over previous
"""Trainium2 Bass kernel for DOAModel correlation + dual-softmax + top-k.

Per batch (reference semantics, no softmax max-subtraction needed since
|logits| <= alpha * max|corr| which is small):
  a    = alpha * l2norm_c(xp)^T l2norm_c(xq)            [N, N]
  x_c  = exp(2a) / (rowsum(exp a)[p] * colsum(exp a)[q])
       = (h1[p] * h2[q] * exp(a))^2,  h = rsqrt(sum exp a)
  valp/valq = top-20 of rows / cols of x_c (sorted), plus x_c and x_c^T.

Implementation notes:
  - matmul in float32r (tf32-like, 1 cyc/row for N>=256): rel err ~1e-4
  - exp on ScalarE with per-partition scale (folds rp) and accum_out
    giving row sums for free; E stored bf16
  - E' = E * h1 on GPSIMD; col sums via one-shot PE matmuls + DVE adds
  - x_c = Square(E' (x) bcast(h2)); x_c^T tiles via PE transpose of E'
    blocks then Square(scale=h2-chunk)
  - top-20 = 3x nc.vector.max + 2x nc.vector.match_replace per tile
Sharding: data-parallel over batch; 16 batches -> 8 cores x 2.
"""

import sys

sys.path.insert(0, "/opt/trn_rl_repo")

import numpy as np

import concourse.bacc as bacc
import concourse.mybir as mybir
import concourse.tile as tile
from concourse.masks import make_identity
from concourse.bass_utils import run_bass_kernel_spmd

dt = mybir.dt
AF = mybir.ActivationFunctionType
ALU = mybir.AluOpType
AX = mybir.AxisListType

C = 896           # channels (contraction dim)
KC = C // 128     # 7 k-chunks
N = 1600          # h*w
NT = 13           # row tiles: 12 x 128 + 1 x 64
NCH = 400         # matmul free-dim chunk (<=512 per matmul, >=256 keeps f32r fast)
ECH = 800         # exp chunk (2 PSUM banks; 2 matmul sub-chunks)
ECHN = N // ECH   # 2
NCHN = N // NCH   # 4
TOPK = 20
NEG = -1.0        # match_replace fill; x_c >= 0

M_OFF = [i * 128 for i in range(NT)]
M_SZ = [128] * 12 + [64]


def _ms(i):
    return slice(M_OFF[i], M_OFF[i] + M_SZ[i])


def build(alpha: float, n_batches: int):
    nc = bacc.Bacc("TRN2", target_bir_lowering=False, debug=False)
    B = n_batches
    xp_d = nc.declare_dram_parameter("xp", [B, C, N], dt.float32, isOutput=False)
    xq_d = nc.declare_dram_parameter("xq", [B, C, N], dt.float32, isOutput=False)
    valp_d = nc.declare_dram_parameter("valp", [B, TOPK, N], dt.float32, isOutput=True)
    valq_d = nc.declare_dram_parameter("valq", [B, TOPK, N], dt.float32, isOutput=True)
    xcoq_d = nc.declare_dram_parameter("xc_o_q", [B, N, N], dt.float32, isOutput=True)
    xcop_d = nc.declare_dram_parameter("xc_o_p", [B, N, N], dt.float32, isOutput=True)

    from contextlib import ExitStack

    with tile.TileContext(nc) as tc, ExitStack() as ctx:
        p_const = ctx.enter_context(tc.tile_pool(name="const", bufs=1))
        p_f32 = ctx.enter_context(tc.tile_pool(name="f32s", bufs=17))
        p_b16 = ctx.enter_context(tc.tile_pool(name="b16s", bufs=25))
        p_sqq = ctx.enter_context(tc.tile_pool(name="sqq", bufs=3))
        p_row = ctx.enter_context(tc.tile_pool(name="rows", bufs=1))
        p_sml = ctx.enter_context(tc.tile_pool(name="sml", bufs=3))
        ps_a = ctx.enter_context(tc.tile_pool(name="psA", bufs=4, space="PSUM"))
        ps_t = ctx.enter_context(tc.tile_pool(name="psT", bufs=2, space="PSUM"))
        ps_r = ctx.enter_context(tc.tile_pool(name="psR", bufs=1, space="PSUM"))
        ps_s = ctx.enter_context(tc.tile_pool(name="psS", bufs=1, space="PSUM"))

        identb = p_const.tile([128, 128], dt.bfloat16)
        make_identity(nc, identb[:])
        identf = p_const.tile([128, 128], dt.float32)
        make_identity(nc, identf[:])
        ones_b = p_const.tile([128, 1], dt.bfloat16)
        nc.vector.memset(ones_b[:], 1.0)

        for b in range(B):
            # =========================================================
            # Phase 0: load, norms, Bq, scale xq in place
            # =========================================================
            xp_t = []
            for k in range(KC):
                t = p_f32.tile([128, N], dt.float32r, tag="big")
                nc.sync.dma_start(
                    t[:], xp_d[b, k * 128:(k + 1) * 128, :].bitcast(dt.float32r))
                xp_t.append(t)
            xq_t = []
            for k in range(KC):
                t = p_f32.tile([128, N], dt.float32, tag="big")
                nc.sync.dma_start(t[:], xq_d[b, k * 128:(k + 1) * 128, :])
                xq_t.append(t)

            # --- sumsq_p chunk-form [128, NT]: one-shot matmuls + SBUF acc
            ssp = p_sml.tile([128, 16], dt.float32, tag="sm_a")
            nc.vector.memset(ssp[:], 0.0)
            for k in range(KC):
                sq = p_b16.tile([128, N], dt.bfloat16, tag="bb")
                nc.scalar.activation(sq[:], xp_t[k][:].bitcast(dt.float32), AF.Square)
                ps = ps_s.tile([128, 16], dt.float32, tag="small")
                for m in range(NT):
                    nc.tensor.matmul(ps[:M_SZ[m], m:m + 1], sq[:, _ms(m)], ones_b[:],
                                     start=(m == 0), stop=(m == NT - 1))
                nc.vector.tensor_tensor(ssp[:, :NT], ssp[:, :NT], ps[:, :NT], ALU.add)
            rp_r = p_sml.tile([128, 16], dt.float32, tag="sm_b")
            nc.vector.reciprocal(rp_r[:, :NT], ssp[:, :NT])
            rpc = p_sml.tile([128, 16], dt.float32, tag="sm_c")
            nc.scalar.activation(rpc[:, :NT], rp_r[:, :NT], AF.Sqrt)

            # --- sumsq_q row-form [1, N] (for Bq): ones^T @ xq^2
            rq_row = p_row.tile([1, N], dt.float32, tag="row_f")
            for ch in range(NCHN):
                ssq = ps_r.tile([1, NCH], dt.float32, tag="rowp")
                for k in range(KC):
                    sq = p_sqq.tile([128, NCH], dt.bfloat16, tag="sqq")
                    nc.scalar.activation(
                        sq[:], xq_t[k][:, ch * NCH:(ch + 1) * NCH], AF.Square)
                    nc.tensor.matmul(ssq[:], ones_b[:], sq[:],
                                     start=(k == 0), stop=(k == KC - 1))
                rec = p_sml.tile([1, NCH], dt.float32, tag="sm_d")
                nc.vector.reciprocal(rec[:], ssq[:])
                # alpha * rsqrt(s) = Sqrt(alpha^2 / s)
                nc.scalar.activation(rq_row[:, ch * NCH:(ch + 1) * NCH], rec[:],
                                     AF.Sqrt, scale=float(alpha) * float(alpha))

            bq = p_f32.tile([128, N], dt.float32, tag="big")
            nc.gpsimd.partition_broadcast(bq[:], rq_row[:])
            # xqs = xq * Bq, rounded to f32r (gpsimd); xq slot recycles
            xqs_t = []
            for k in range(KC):
                t = p_f32.tile([128, N], dt.float32r, tag="big")
                nc.vector.tensor_tensor(t[:], xq_t[k][:], bq[:], ALU.mult)
                xqs_t.append(t)

            # =========================================================
            # Phase 1: per row-tile matmul + exp + E' + col sums
            # =========================================================
            ep_t = []
            s2sb = p_sml.tile([128, 16], dt.float32, tag="sm_e")
            nc.vector.memset(s2sb[:], 0.0)
            s1acc = p_sml.tile([128, 64], dt.float32, tag="sm_f")
            for m in range(NT):
                msz = M_SZ[m]
                e_t = p_b16.tile([128, N], dt.bfloat16, tag="bb")
                if msz < 128:
                    nc.vector.memset(e_t[:], 0.0)
                for ch in range(NCHN):
                    pa = ps_a.tile([128, NCH], dt.float32, tag="A")
                    for k in range(KC):
                        nc.tensor.matmul(
                            pa[:msz, :],
                            xp_t[k][:, _ms(m)],
                            xqs_t[k][:, ch * NCH:(ch + 1) * NCH],
                            start=(k == 0), stop=(k == KC - 1))
                    nc.scalar.activation(
                        e_t[:msz, ch * NCH:(ch + 1) * NCH], pa[:msz, :], AF.Exp,
                        scale=rpc[:msz, m:m + 1],
                        accum_out=s1acc[:msz, 4 * m + ch:4 * m + ch + 1])
                ep_t.append(e_t)

                # col-sum contribution of this row tile (one-shot + SBUF acc)
                ps2 = ps_s.tile([128, 16], dt.float32, tag="small")
                for j in range(NT):
                    nc.tensor.matmul(ps2[:M_SZ[j], j:j + 1], e_t[:, _ms(j)], ones_b[:],
                                     start=(j == 0), stop=(j == NT - 1))
                nc.vector.tensor_tensor(s2sb[:, :NT], s2sb[:, :NT], ps2[:, :NT], ALU.add)

            # =========================================================
            # Phase 1b: batched h1 (rows) + h2 (cols), one table load each
            # =========================================================
            s1 = p_sml.tile([128, 16], dt.float32, tag="sm_g")
            nc.vector.reduce_sum(s1[:, :NT],
                                 s1acc[:, :4 * NT].rearrange("p (m c) -> p m c", c=4),
                                 axis=AX.X)
            h1r = p_sml.tile([128, 16], dt.float32, tag="sm_h")
            nc.vector.reciprocal(h1r[:, :NT], s1[:, :NT])
            h1c = p_sml.tile([128, 16], dt.float32, tag="sm_i")
            nc.scalar.activation(h1c[:, :NT], h1r[:, :NT], AF.Sqrt)
            h2r_ = p_sml.tile([128, 16], dt.float32, tag="sm_j")
            nc.vector.reciprocal(h2r_[:, :NT], s2sb[:, :NT])
            h2c = p_sml.tile([128, 16], dt.float32, tag="sm_k")
            nc.scalar.activation(h2c[:, :NT], h2r_[:, :NT], AF.Sqrt)
            h2row = p_row.tile([1, N], dt.bfloat16, tag="row_b")
            for j in range(NT):
                pt = ps_t.tile([1, 128], dt.float32, tag="T")
                nc.tensor.transpose(pt[:, :M_SZ[j]], h2c[:M_SZ[j], j:j + 1],
                                    identf[:M_SZ[j], :M_SZ[j]])
                nc.vector.tensor_copy(h2row[:, _ms(j)], pt[:, :M_SZ[j]])
            bs2h = p_b16.tile([128, N], dt.bfloat16, tag="bb")
            nc.gpsimd.partition_broadcast(bs2h[:], h2row[:])

            # Phase 1c: E' = E * h1 in place (DVE tensor_scalar, bf16 4x)
            for m in range(NT):
                nc.vector.tensor_scalar_mul(ep_t[m][:M_SZ[m], :], ep_t[m][:M_SZ[m], :],
                                            h1c[:M_SZ[m], m:m + 1])

            # =========================================================
            # Phase 2b: x_c tiles (rows) and x_c^T tiles (cols) + top-k
            # =========================================================
            valp_row = p_f32.tile([32, N], dt.float32, tag="big")
            valq_row = p_f32.tile([32, N], dt.float32, tag="big")

            def topk_into(xc, msz, vrow, m):
                vals = p_sml.tile([128, 24], dt.bfloat16, tag="sm_l")
                valf = p_sml.tile([128, 24], dt.float32, tag="sm_m")
                scr = p_b16.tile([128, N], dt.bfloat16, tag="bb")
                scr2 = p_b16.tile([128, N], dt.bfloat16, tag="bb")
                nc.vector.max(out=vals[:msz, 0:8], in_=xc[:msz, :])
                nc.vector.match_replace(out=scr[:msz, :], in_to_replace=vals[:msz, 0:8],
                                        in_values=xc[:msz, :], imm_value=NEG)
                nc.vector.max(out=vals[:msz, 8:16], in_=scr[:msz, :])
                nc.vector.match_replace(out=scr2[:msz, :], in_to_replace=vals[:msz, 8:16],
                                        in_values=scr[:msz, :], imm_value=NEG)
                nc.vector.max(out=vals[:msz, 16:24], in_=scr2[:msz, :])
                nc.vector.tensor_copy(valf[:msz, :], vals[:msz, :])
                pt = ps_t.tile([32, 128], dt.float32, tag="T")
                nc.tensor.transpose(pt[:TOPK, :msz], valf[:msz, :TOPK],
                                    identf[:msz, :msz])
                nc.vector.tensor_copy(vrow[:TOPK, _ms(m)], pt[:TOPK, :msz])

            for t in range(NT):
                msz = M_SZ[t]
                # rows side: x_c tile t
                g = p_b16.tile([128, N], dt.bfloat16, tag="bb")
                nc.vector.tensor_tensor(g[:msz, :], ep_t[t][:msz, :], bs2h[:msz, :],
                                        ALU.mult)
                xc = p_b16.tile([128, N], dt.bfloat16, tag="bb")
                nc.scalar.activation(xc[:msz, :], g[:msz, :], AF.Square)
                nc.gpsimd.dma_start(xcoq_d[b, _ms(t), :], xc[:msz, :])
                topk_into(xc, msz, valp_row, t)

                # cols side: x_c^T tile t from 13 transposed blocks
                xct = p_b16.tile([128, N], dt.bfloat16, tag="bb")
                for i in range(NT):
                    pt = ps_t.tile([128, 128], dt.bfloat16, tag="T")
                    nc.tensor.transpose(pt[:msz, :], ep_t[i][:, _ms(t)], identb[:])
                    nc.scalar.activation(xct[:msz, _ms(i)], pt[:msz, :M_SZ[i]],
                                         AF.Square, scale=h2c[:msz, t:t + 1])
                nc.gpsimd.dma_start(xcop_d[b, _ms(t), :], xct[:msz, :])
                topk_into(xct, msz, valq_row, t)

            nc.sync.dma_start(valp_d[b], valp_row[:TOPK, :])
            nc.sync.dma_start(valq_d[b], valq_row[:TOPK, :])

    nc.compile()
    return nc


_CACHE = {}


def _get_kernel(alpha: float, n_batches: int):
    key = (round(float(alpha), 6), n_batches)
    if key not in _CACHE:
        _CACHE[key] = build(float(alpha), n_batches)
    return _CACHE[key]


def kernel(xp, xq, alpha):
    xp = np.asarray(xp, dtype=np.float32)
    xq = np.asarray(xq, dtype=np.float32)
    b, c, h1, w1 = xp.shape
    _, _, h2, w2 = xq.shape
    n_cores = 8
    assert b % n_cores == 0
    bpc = b // n_cores
    nc = _get_kernel(float(alpha), bpc)

    xp_f = xp.reshape(b, c, h1 * w1)
    xq_f = xq.reshape(b, c, h2 * w2)
    in_maps = [
        {"xp": xp_f[i * bpc:(i + 1) * bpc], "xq": xq_f[i * bpc:(i + 1) * bpc],
         }
        for i in range(n_cores)
    ]
    res = run_bass_kernel_spmd(nc, in_maps, list(range(n_cores)))
    valp = np.concatenate([r["valp"] for r in res.results], axis=0)
    valq = np.concatenate([r["valq"] for r in res.results], axis=0)
    xcop = np.concatenate([r["xc_o_p"] for r in res.results], axis=0)
    xcoq = np.concatenate([r["xc_o_q"] for r in res.results], axis=0)
    return (
        valp.reshape(b, TOPK, h1, w1),
        valq.reshape(b, TOPK, h2, w2),
        xcop.reshape(b, h2 * w2, h1, w1),
        xcoq.reshape(b, h1 * w1, h2, w2),
    )
